# revision 2
# baseline (speedup 1.0000x reference)
"""Trainium2 Bass kernel for the dense_cnn problem:

    t1 = conv1x1(x, w1); t2 = gelu(t1)
    t5 = dwconv5x5(t2, w5, pad=2)
    t6 = dwconv7x7_dil3(t5, w6, pad=9)
    t7 = conv1x1(t6, w7); t8 = t7 * t2; t9 = conv1x1(t8, w9)
    out = x + t9

Sharding: data-parallel over batch N=32 across 8 cores (4 samples/core).

Per-core schedule (fp16 datapath, fp32 PSUM accumulation):
  - x shipped as fp16 (residual also read back in fp16).
  - 1x1 convs: dense PE matmuls (lhsT = W.T in fp16), 7 chunks of 8 rows
    (N=448) per 128-channel group.
  - depthwise convs, split per tap between PE and DVE:
      PE taps: diagonal-matrix matmuls accumulating in PSUM; the scalar
      engine evicts the PSUM partial sum into the conv's output buffer.
      DVE taps: fused scalar_tensor_tensor FMAs (out = src*w + out)
      added in-place on top of the evicted partial sum. Padded buffers
      are laid out so every DVE operand view is 4-byte aligned (2x mode).
  - dw5: 14 PE taps (all odd-dx + 4 even), 11 DVE taps (even dx).
    dw7: 28 PE taps (even jx), 21 DVE taps (odd jx).
"""

import numpy as np

import concourse.bass as bass
import concourse.mybir as mybir
from concourse.tile import TileContext
from concourse.bass_utils import run_bass_kernel_spmd

# ---------------------------------------------------------------------------
# Workaround: this walrus build rejects >N sem waits on the TileContext tail
# drain ("Too many sync wait commands"). Split them one-per-drain.
from concourse.vector_clock import ScopedClock, VectorClock


def _drain_and_barrier_split(self, tick_clock, wait_clock):
    vc = tick_clock.global_clock
    for proc in range(len(vc)):
        tick = vc[proc]
        if tick <= 0:
            continue
        d = self.nc.sync.drain()
        req = ScopedClock({None: VectorClock([0] * len(vc))})
        req.require_at_least(None, proc, tick)
        wait_clock.add_sem_waits(d.ins, req)
    self.nc.all_engine_barrier()
    assert self.sems is not None
    popped = self.nc._tile_sem_poison_stack.pop()
    assert popped is self._sem_poison
    self.nc.clear_and_free_semaphores(list(self.sems.allocated().values()))
    self.nc.all_engine_barrier()


TileContext._drain_and_barrier = _drain_and_barrier_split

# This walrus build also rejects >1 sem wait on regular engine instructions.
# Post-process the serialized BIR: hoist excess waits onto same-engine NoOps
# inserted right before the instruction (engines execute block instructions
# in order, so waiting earlier on the same engine is equivalent).
import json as _json

_orig_to_json_bytes = bass.Bass.to_json_bytes


def _to_json_bytes_split_waits(self):
    d = _json.loads(_orig_to_json_bytes(self))
    ctr = 0
    for fn in d.get("functions", []):
        for blk in fn.get("blocks", []):
            insts = blk.get("instructions", [])
            out = []
            for inst in insts:
                si = inst.get("sync_info")
                waits = (si or {}).get("on_wait") or []
                if len(waits) > 1:
                    for w in waits[:-1]:
                        out.append({
                            "debug": inst.get("debug", 0),
                            "engine": inst["engine"],
                            "ins": [],
                            "outs": [],
                            "name": f"{inst['name']}_hw{ctr}",
                            "opcode": "NoOp",
                            "sync_info": {"on_wait": [w], "on_update": []},
                        })
                        ctr += 1
                    si["on_wait"] = waits[-1:]
                out.append(inst)
            blk["instructions"] = out
    return _json.dumps(d).encode()


bass.Bass.to_json_bytes = _to_json_bytes_split_waits
# ---------------------------------------------------------------------------

F16 = mybir.dt.float16
F32 = mybir.dt.float32
AF = mybir.ActivationFunctionType
OP = mybir.AluOpType

N_CORES = 8
NS = 4              # samples per core
C, H, W = 384, 56, 56
G = 3               # channel groups of 128
HW = H * W          # 3136
# t2 padded buffer: pad 2 on all sides -> [60, 60]
T2H, T2W = 60, 60
# t5 padded buffer: pad 9 rows top/bottom, 10 cols left (so the interior
# starts on an even element offset -> 4B-aligned fp16 DVE views), width
# padded to 76 so the row stride stays even.
T5H, T5W = 74, 76
T5R0, T5C0 = 9, 10  # interior origin
CH_ROWS = 8         # output rows per PSUM chunk
NCH = H // CH_ROWS  # 7 chunks
CHF = CH_ROWS * W   # 448 free elems per chunk
DV_ROWS = 28        # output rows per DVE half
NDV = H // DV_ROWS  # 2

# Depthwise tap assignment.
# dw5 view (into t2pad): out(y,x) += w[dy,dx] * t2pad[y+dy, x+dx]
#   DVE needs even dx (aligned); PE takes the 10 odd-dx taps + 4 even.
# dw7 view (into t5pad): out(y,x) += w[jy,jx] * t5pad[y+3jy, 1+3jx + x]
#   col offset 1+3jx is even iff jx is odd -> DVE takes odd jx (21),
#   PE takes even jx (28).
_EVEN5 = [(dy, dx) for dy in range(5) for dx in range(5) if dx % 2 == 0]
_ODD5 = [(dy, dx) for dy in range(5) for dx in range(5) if dx % 2 == 1]
PE5 = _ODD5 + _EVEN5[:4]
DVE5 = _EVEN5[4:]
PE7 = [(jy, jx) for jy in range(7) for jx in range(7) if jx % 2 == 0]
DVE7 = [(jy, jx) for jy in range(7) for jx in range(7) if jx % 2 == 1]
assert len(PE5) + len(DVE5) == 25 and len(PE7) + len(DVE7) == 49


def _build_program(n_loop=1):
    nc = bass.Bass("TRN2", target_bir_lowering=False, debug=False)

    x_d = nc.dram_tensor("x16", [NS, G, 128, HW], F16, kind="ExternalInput")
    w1T_d = nc.dram_tensor("w1T", [G, 128, C], F16, kind="ExternalInput")
    w7T_d = nc.dram_tensor("w7T", [G, 128, C], F16, kind="ExternalInput")
    w9T_d = nc.dram_tensor("w9T", [G, 128, C], F16, kind="ExternalInput")
    w5t_d = nc.dram_tensor("w5t", [G, 128, 25], F32, kind="ExternalInput")
    w6t_d = nc.dram_tensor("w6t", [G, 128, 49], F32, kind="ExternalInput")
    id_d = nc.dram_tensor("ident", [128, 128], F16, kind="ExternalInput")
    o_d = nc.dram_tensor("out", [NS, G, 128, HW], F32, kind="ExternalOutput")

    with TileContext(nc) as tc:
        with (
            tc.tile_pool(name="const", bufs=1) as const,
            tc.tile_pool(name="big16", bufs=9) as big16,
            tc.tile_pool(name="pads", bufs=1) as pads,
            tc.tile_pool(name="small", bufs=4) as small_p,
            tc.tile_pool(name="psum", bufs=8, space="PSUM") as pp,
        ):
            # ---- constants -------------------------------------------------
            w1T = [const.tile([128, C], F16, name=f"w1T{k}") for k in range(G)]
            w7T = [const.tile([128, C], F16, name=f"w7T{k}") for k in range(G)]
            w9T = [const.tile([128, C], F16, name=f"w9T{k}") for k in range(G)]
            w5t = [const.tile([128, 25], F32, name=f"w5t{g}") for g in range(G)]
            w6t = [const.tile([128, 49], F32, name=f"w6t{g}") for g in range(G)]
            ident = const.tile([128, 128], F16, name="ident")
            for k in range(G):
                nc.sync.dma_start(out=w1T[k][:], in_=w1T_d.ap()[k])
                nc.sync.dma_start(out=w7T[k][:], in_=w7T_d.ap()[k])
                nc.sync.dma_start(out=w9T[k][:], in_=w9T_d.ap()[k])
                nc.sync.dma_start(out=w5t[k][:], in_=w5t_d.ap()[k])
                nc.sync.dma_start(out=w6t[k][:], in_=w6t_d.ap()[k])
            nc.sync.dma_start(out=ident[:], in_=id_d.ap())

            # Diagonal weight matrices for PE depthwise taps.
            diag5 = {}
            for g in range(G):
                for (dy, dx) in PE5:
                    t = const.tile([128, 128], F16, name=f"d5_{g}_{dy}_{dx}")
                    nc.vector.tensor_scalar_mul(
                        t[:], ident[:], w5t[g][:, 5 * dy + dx : 5 * dy + dx + 1]
                    )
                    diag5[(g, dy, dx)] = t
            diag6 = {}
            for g in range(G):
                for (jy, jx) in PE7:
                    t = const.tile([128, 128], F16, name=f"d6_{g}_{jy}_{jx}")
                    nc.vector.tensor_scalar_mul(
                        t[:], ident[:], w6t[g][:, 7 * jy + jx : 7 * jy + jx + 1]
                    )
                    diag6[(g, jy, jx)] = t

            # ---- padded scratch (zero margins persist across samples) ------
            t2pad = [pads.tile([128, T2H * T2W], F16, name=f"t2p{g}") for g in range(G)]
            t5pad = [pads.tile([128, T5H * T5W], F16, name=f"t5p{g}") for g in range(G)]
            for g in range(G):
                nc.gpsimd.memset(t2pad[g][:], 0.0)
                nc.gpsimd.memset(t5pad[g][:], 0.0)
            t2p3 = [t.rearrange("p (h w) -> p h w", w=T2W) for t in t2pad]
            t5p3 = [t.rearrange("p (h w) -> p h w", w=T5W) for t in t5pad]

            # ---- per-sample program ---------------------------------------
            import contextlib

            loop_cm = (
                tc.For_i(0, n_loop, 1) if n_loop > 1 else contextlib.nullcontext()
            )
            with loop_cm:
                _emit_samples(nc, tc, locals())
    return nc


def _emit_samples(nc, tc, env):
    (x_d, o_d) = (env["x_d"], env["o_d"])
    (w1T, w7T, w9T, w5t, w6t) = (
        env["w1T"], env["w7T"], env["w9T"], env["w5t"], env["w6t"]
    )
    (diag5, diag6) = (env["diag5"], env["diag6"])
    (t2p3, t5p3) = (env["t2p3"], env["t5p3"])
    (big16, small_p, pp) = (env["big16"], env["small_p"], env["pp"])
    for n in range(NS):
        # A) load x (already fp16)
        x16 = []
        for g in range(G):
            xt = big16.tile([128, HW], F16, name=f"x16_{n}_{g}", tag="b16")
            nc.sync.dma_start(out=xt[:], in_=x_d.ap()[n, g])
            x16.append(xt)

        # B) t1 = w1 @ x ; t2 = gelu(t1) -> t2pad interior
        for m in range(G):
            for ch in range(NCH):
                ps = pp.tile([128, CHF], F32, name=f"psB{n}{m}{ch}", tag="ps")
                for k in range(G):
                    nc.tensor.matmul(
                        ps[:],
                        w1T[k][:, 128 * m : 128 * (m + 1)],
                        x16[k][:, CHF * ch : CHF * (ch + 1)],
                        start=(k == 0),
                        stop=(k == G - 1),
                    )
                nc.scalar.activation(
                    t2p3[m][:, 2 + CH_ROWS * ch : 2 + CH_ROWS * (ch + 1), 2 : 2 + W],
                    ps[:],
                    AF.Gelu,
                )

        # C) t5 = dw5(t2) -> t5pad interior.
        #    PE taps accumulate in PSUM; ACT evicts into t5pad; DVE taps
        #    then FMA in place on the interior.
        for g in range(G):
            for ch in range(NCH):
                r0 = CH_ROWS * ch
                ps = pp.tile([128, CHF], F32, name=f"psC{n}{g}{ch}", tag="ps")
                for i, (dy, dx) in enumerate(PE5):
                    nc.tensor.matmul(
                        ps[:],
                        diag5[(g, dy, dx)][:],
                        t2p3[g][:, r0 + dy : r0 + dy + CH_ROWS, dx : dx + W],
                        start=(i == 0),
                        stop=(i == len(PE5) - 1),
                    )
                nc.scalar.activation(
                    t5p3[g][:, T5R0 + r0 : T5R0 + r0 + CH_ROWS, T5C0 : T5C0 + W],
                    ps[:],
                    AF.Copy,
                )
            for d in range(NDV):
                r0 = DV_ROWS * d
                outv = t5p3[g][:, T5R0 + r0 : T5R0 + r0 + DV_ROWS, T5C0 : T5C0 + W]
                for (dy, dx) in DVE5:
                    nc.vector.scalar_tensor_tensor(
                        outv,
                        t2p3[g][:, r0 + dy : r0 + dy + DV_ROWS, dx : dx + W],
                        w5t[g][:, 5 * dy + dx : 5 * dy + dx + 1],
                        outv,
                        op0=OP.mult,
                        op1=OP.add,
                    )

        # D) t6 = dw7_dil3(t5) -> dense t6 tiles (same PE/ACT/DVE scheme)
        t6 = []
        for g in range(G):
            t6g = big16.tile([128, HW], F16, name=f"t6_{n}_{g}", tag="b16")
            t6g3 = t6g.rearrange("p (h w) -> p h w", w=W)
            for ch in range(NCH):
                r0 = CH_ROWS * ch
                ps = pp.tile([128, CHF], F32, name=f"psD{n}{g}{ch}", tag="ps")
                for i, (jy, jx) in enumerate(PE7):
                    nc.tensor.matmul(
                        ps[:],
                        diag6[(g, jy, jx)][:],
                        t5p3[g][:, r0 + 3 * jy : r0 + 3 * jy + CH_ROWS,
                                1 + 3 * jx : 1 + 3 * jx + W],
                        start=(i == 0),
                        stop=(i == len(PE7) - 1),
                    )
                nc.scalar.activation(
                    t6g3[:, r0 : r0 + CH_ROWS, :], ps[:], AF.Copy
                )
            for d in range(NDV):
                r0 = DV_ROWS * d
                outv = t6g3[:, r0 : r0 + DV_ROWS, :]
                for (jy, jx) in DVE7:
                    nc.vector.scalar_tensor_tensor(
                        outv,
                        t5p3[g][:, r0 + 3 * jy : r0 + 3 * jy + DV_ROWS,
                                1 + 3 * jx : 1 + 3 * jx + W],
                        w6t[g][:, 7 * jy + jx : 7 * jy + jx + 1],
                        outv,
                        op0=OP.mult,
                        op1=OP.add,
                    )
            t6.append(t6g)

        # E) t7 = w7 @ t6 ; t8 = t7 * t2 (in place)
        t8 = []
        for m in range(G):
            t7m = big16.tile([128, HW], F16, name=f"t7_{n}_{m}", tag="b16")
            for ch in range(NCH):
                ps = pp.tile([128, CHF], F32, name=f"psE{n}{m}{ch}", tag="ps")
                for k in range(G):
                    nc.tensor.matmul(
                        ps[:],
                        w7T[k][:, 128 * m : 128 * (m + 1)],
                        t6[k][:, CHF * ch : CHF * (ch + 1)],
                        start=(k == 0),
                        stop=(k == G - 1),
                    )
                nc.scalar.activation(
                    t7m[:, CHF * ch : CHF * (ch + 1)], ps[:], AF.Copy
                )
            t7m3 = t7m.rearrange("p (h w) -> p h w", w=W)
            nc.vector.tensor_tensor(
                t7m3[:],
                t7m3[:],
                t2p3[m][:, 2 : 2 + H, 2 : 2 + W],
                OP.mult,
            )
            t8.append(t7m)

        # F) t9 = w9 @ t8 ; out = x + t9 (residual read back in fp16)
        for m in range(G):
            for ch in range(NCH):
                ps = pp.tile([128, CHF], F32, name=f"psF{n}{m}{ch}", tag="ps")
                for k in range(G):
                    nc.tensor.matmul(
                        ps[:],
                        w9T[k][:, 128 * m : 128 * (m + 1)],
                        t8[k][:, CHF * ch : CHF * (ch + 1)],
                        start=(k == 0),
                        stop=(k == G - 1),
                    )
                res = small_p.tile([128, CHF], F16, name=f"rs{n}{m}{ch}", tag="res")
                nc.sync.dma_start(
                    out=res[:], in_=x_d.ap()[n, m, :, CHF * ch : CHF * (ch + 1)]
                )
                ost = small_p.tile([128, CHF], F32, name=f"os{n}{m}{ch}", tag="ost")
                nc.vector.tensor_tensor(ost[:], ps[:], res[:], OP.add)
                nc.sync.dma_start(
                    out=o_d.ap()[n, m, :, CHF * ch : CHF * (ch + 1)], in_=ost[:]
                )


_NC_CACHE = None


def _get_nc():
    global _NC_CACHE
    if _NC_CACHE is None:
        _NC_CACHE = _build_program()
    return _NC_CACHE


def _prep_shared_inputs(w1, w5, w6, w7, w9):
    def lhsT(w):
        return np.ascontiguousarray(np.asarray(w, np.float32).T).astype(np.float16).reshape(G, 128, C)

    return {
        "w1T": lhsT(w1),
        "w7T": lhsT(w7),
        "w9T": lhsT(w9),
        "w5t": np.asarray(w5, np.float32).reshape(C, 25).reshape(G, 128, 25),
        "w6t": np.asarray(w6, np.float32).reshape(C, 49).reshape(G, 128, 49),
        "ident": np.eye(128, dtype=np.float16),
    }


def _make_in_maps(x, w1, w5, w6, w7, w9):
    x16 = np.asarray(x, np.float32).astype(np.float16)
    N = x16.shape[0]
    assert N == N_CORES * NS
    shared = _prep_shared_inputs(w1, w5, w6, w7, w9)
    xs = x16.reshape(N_CORES, NS, G, 128, HW)
    return [{"x16": np.ascontiguousarray(xs[i]), **shared} for i in range(N_CORES)]


def kernel(x, w1, w5, w6, w7, w9, _trace=False, _tmpdir=None):
    in_maps = _make_in_maps(x, w1, w5, w6, w7, w9)
    nc = _get_nc()
    res = run_bass_kernel_spmd(
        nc, in_maps, core_ids=list(range(N_CORES)), trace=_trace, tmpdir=_tmpdir
    )
    outs = [res.results[i]["out"] for i in range(N_CORES)]
    out = np.stack(outs, axis=0).reshape(x.shape[0], C, H, W)
    if _trace:
        kernel.last_exec_time_ns = res.exec_time_ns
        kernel.last_results = res
    return out


# revision 6
# speedup vs baseline: 1.3492x; 1.3492x over previous
"""Trainium2 Bass kernel for the dense_cnn problem:

    t1 = conv1x1(x, w1); t2 = gelu(t1)
    t5 = dwconv5x5(t2, w5, pad=2)
    t6 = dwconv7x7_dil3(t5, w6, pad=9)
    t7 = conv1x1(t6, w7); t8 = t7 * t2; t9 = conv1x1(t8, w9)
    out = x + t9

Sharding: data-parallel over batch N=32 across 8 cores (4 samples/core).

Per-core schedule (fp16 datapath, fp32 PSUM accumulation):
  - x shipped as fp16 (residual also read back in fp16).
  - 1x1 convs: dense PE matmuls (lhsT = W.T in fp16), 7 chunks of 8 rows
    (N=448) per 128-channel group.
  - depthwise convs, split per tap between PE and DVE:
      PE taps: diagonal-matrix matmuls accumulating in PSUM; the scalar
      engine evicts the PSUM partial sum into the conv's output buffer.
      DVE taps: fused scalar_tensor_tensor FMAs (out = src*w + out)
      added in-place on top of the evicted partial sum. Padded buffers
      are laid out so every DVE operand view is 4-byte aligned (2x mode).
  - dw5: 14 PE taps (all odd-dx + 4 even), 11 DVE taps (even dx).
    dw7: 28 PE taps (even jx), 21 DVE taps (odd jx).
"""

import numpy as np

import concourse.bass as bass
import concourse.mybir as mybir
from concourse.tile import TileContext
from concourse.bass_utils import run_bass_kernel_spmd

# ---------------------------------------------------------------------------
# Workaround: this walrus build rejects >N sem waits on the TileContext tail
# drain ("Too many sync wait commands"). Split them one-per-drain.
from concourse.vector_clock import ScopedClock, VectorClock


def _drain_and_barrier_split(self, tick_clock, wait_clock):
    vc = tick_clock.global_clock
    for proc in range(len(vc)):
        tick = vc[proc]
        if tick <= 0:
            continue
        d = self.nc.sync.drain()
        req = ScopedClock({None: VectorClock([0] * len(vc))})
        req.require_at_least(None, proc, tick)
        wait_clock.add_sem_waits(d.ins, req)
    self.nc.all_engine_barrier()
    assert self.sems is not None
    popped = self.nc._tile_sem_poison_stack.pop()
    assert popped is self._sem_poison
    self.nc.clear_and_free_semaphores(list(self.sems.allocated().values()))
    self.nc.all_engine_barrier()


TileContext._drain_and_barrier = _drain_and_barrier_split

# This walrus build also rejects >1 sem wait on regular engine instructions.
# Post-process the serialized BIR: hoist excess waits onto same-engine NoOps
# inserted right before the instruction (engines execute block instructions
# in order, so waiting earlier on the same engine is equivalent).
import json as _json

_orig_to_json_bytes = bass.Bass.to_json_bytes


def _to_json_bytes_split_waits(self):
    d = _json.loads(_orig_to_json_bytes(self))
    ctr = 0
    for fn in d.get("functions", []):
        for blk in fn.get("blocks", []):
            insts = blk.get("instructions", [])
            out = []
            for inst in insts:
                si = inst.get("sync_info")
                waits = (si or {}).get("on_wait") or []
                if len(waits) > 1:
                    for w in waits[:-1]:
                        out.append({
                            "debug": inst.get("debug", 0),
                            "engine": inst["engine"],
                            "ins": [],
                            "outs": [],
                            "name": f"{inst['name']}_hw{ctr}",
                            "opcode": "NoOp",
                            "sync_info": {"on_wait": [w], "on_update": []},
                        })
                        ctr += 1
                    si["on_wait"] = waits[-1:]
                out.append(inst)
            blk["instructions"] = out
    return _json.dumps(d).encode()


bass.Bass.to_json_bytes = _to_json_bytes_split_waits
# ---------------------------------------------------------------------------

F16 = mybir.dt.float16
F32 = mybir.dt.float32
AF = mybir.ActivationFunctionType
OP = mybir.AluOpType

N_CORES = 8
NS = 4              # samples per core
C, H, W = 384, 56, 56
G = 3               # channel groups of 128
HW = H * W          # 3136
# t2 padded buffer: pad 2 on all sides -> [60, 60]
T2H, T2W = 60, 60
# t5 padded buffer: pad 9 rows top/bottom, 10 cols left (so the interior
# starts on an even element offset -> 4B-aligned fp16 DVE views), width
# padded to 76 so the row stride stays even.
T5H, T5W = 74, 76
T5R0, T5C0 = 9, 10  # interior origin
CH_ROWS = 8         # output rows per PSUM chunk
NCH = H // CH_ROWS  # 7 chunks
CHF = CH_ROWS * W   # 448 free elems per chunk
DV_ROWS = 28        # output rows per DVE half
NDV = H // DV_ROWS  # 2

# Depthwise tap assignment, three lanes:
#   PE:  diagonal-matrix matmuls accumulating in PSUM (weight-stationary,
#        7 chunks per tap).
#   DVE pair: ts_mul (tap -> dense tmp) + tensor_tensor add in place on
#        the conv output. The mul needs a 4B-aligned source view:
#        dw5 -> even dx; dw7 -> odd jx (col offset 1+3jx even).
#   ACT hybrid: scalar-engine copy with per-partition scale (any
#        alignment) -> dense tmp, + DVE add in place.
# dw5 view (into t2pad): out(y,x) += w[dy,dx] * t2pad[y+dy, x+dx]
# dw7 view (into t5pad): out(y,x) += w[jy,jx] * t5pad[y+3jy, 1+3jx + x]
_EVEN5 = [(dy, dx) for dy in range(5) for dx in range(5) if dx % 2 == 0]
_ODD5 = [(dy, dx) for dy in range(5) for dx in range(5) if dx % 2 == 1]
DVE5 = _EVEN5[:8]          # 8 pair taps (aligned)
ACT5 = _EVEN5[8:10] + _ODD5[:3]   # 5 hybrid taps
PE5 = _EVEN5[10:] + _ODD5[3:]     # 12 PE taps
_ODD7 = [(jy, jx) for jy in range(7) for jx in range(7) if jx % 2 == 1]
_EVEN7 = [(jy, jx) for jy in range(7) for jx in range(7) if jx % 2 == 0]
DVE7 = _ODD7[:7]           # 7 pair taps (aligned)
ACT7 = _ODD7[7:14] + _EVEN7[:4]   # 11 hybrid taps
PE7 = _ODD7[14:] + _EVEN7[4:]     # 31 PE taps
assert len(PE5) + len(DVE5) + len(ACT5) == 25
assert len(PE7) + len(DVE7) + len(ACT7) == 49


def _build_program(n_loop=1):
    nc = bass.Bass("TRN2", target_bir_lowering=False, debug=False)

    x_d = nc.dram_tensor("x16", [NS, G, 128, HW], F16, kind="ExternalInput")
    w1T_d = nc.dram_tensor("w1T", [G, 128, C], F16, kind="ExternalInput")
    w7T_d = nc.dram_tensor("w7T", [G, 128, C], F16, kind="ExternalInput")
    w9T_d = nc.dram_tensor("w9T", [G, 128, C], F16, kind="ExternalInput")
    w5t_d = nc.dram_tensor("w5t", [G, 128, 25], F32, kind="ExternalInput")
    w6t_d = nc.dram_tensor("w6t", [G, 128, 49], F32, kind="ExternalInput")
    id_d = nc.dram_tensor("ident", [128, 128], F16, kind="ExternalInput")
    o_d = nc.dram_tensor("out", [NS, G, 128, HW], F32, kind="ExternalOutput")

    with TileContext(nc) as tc:
        with (
            tc.tile_pool(name="const", bufs=1) as const,
            tc.tile_pool(name="big16", bufs=7) as big16,
            tc.tile_pool(name="pads", bufs=1) as pads,
            tc.tile_pool(name="tmps", bufs=6) as tmp_p,
            tc.tile_pool(name="small", bufs=4) as small_p,
            tc.tile_pool(name="psum", bufs=8, space="PSUM") as pp,
        ):
            # ---- constants -------------------------------------------------
            w1T = [const.tile([128, C], F16, name=f"w1T{k}") for k in range(G)]
            w7T = [const.tile([128, C], F16, name=f"w7T{k}") for k in range(G)]
            w9T = [const.tile([128, C], F16, name=f"w9T{k}") for k in range(G)]
            w5t = [const.tile([128, 25], F32, name=f"w5t{g}") for g in range(G)]
            w6t = [const.tile([128, 49], F32, name=f"w6t{g}") for g in range(G)]
            ident = const.tile([128, 128], F16, name="ident")
            for k in range(G):
                nc.sync.dma_start(out=w1T[k][:], in_=w1T_d.ap()[k])
                nc.sync.dma_start(out=w7T[k][:], in_=w7T_d.ap()[k])
                nc.sync.dma_start(out=w9T[k][:], in_=w9T_d.ap()[k])
                nc.sync.dma_start(out=w5t[k][:], in_=w5t_d.ap()[k])
                nc.sync.dma_start(out=w6t[k][:], in_=w6t_d.ap()[k])
            nc.sync.dma_start(out=ident[:], in_=id_d.ap())

            # Diagonal weight matrices for PE depthwise taps.
            diag5 = {}
            for g in range(G):
                for (dy, dx) in PE5:
                    t = const.tile([128, 128], F16, name=f"d5_{g}_{dy}_{dx}")
                    nc.vector.tensor_scalar_mul(
                        t[:], ident[:], w5t[g][:, 5 * dy + dx : 5 * dy + dx + 1]
                    )
                    diag5[(g, dy, dx)] = t
            diag6 = {}
            for g in range(G):
                for (jy, jx) in PE7:
                    t = const.tile([128, 128], F16, name=f"d6_{g}_{jy}_{jx}")
                    nc.vector.tensor_scalar_mul(
                        t[:], ident[:], w6t[g][:, 7 * jy + jx : 7 * jy + jx + 1]
                    )
                    diag6[(g, jy, jx)] = t

            # ---- padded scratch (zero margins persist across samples) ------
            t2pad = [pads.tile([128, T2H * T2W], F16, name=f"t2p{g}") for g in range(G)]
            t5pad = [pads.tile([128, T5H * T5W], F16, name=f"t5p{g}") for g in range(G)]
            for g in range(G):
                nc.gpsimd.memset(t2pad[g][:], 0.0)
                nc.gpsimd.memset(t5pad[g][:], 0.0)
            t2p3 = [t.rearrange("p (h w) -> p h w", w=T2W) for t in t2pad]
            t5p3 = [t.rearrange("p (h w) -> p h w", w=T5W) for t in t5pad]

            # ---- per-sample program ---------------------------------------
            import contextlib

            loop_cm = (
                tc.For_i(0, n_loop, 1) if n_loop > 1 else contextlib.nullcontext()
            )
            with loop_cm:
                _emit_samples(nc, tc, locals())
    return nc


def _emit_samples(nc, tc, env):
    (x_d, o_d) = (env["x_d"], env["o_d"])
    (w1T, w7T, w9T, w5t, w6t) = (
        env["w1T"], env["w7T"], env["w9T"], env["w5t"], env["w6t"]
    )
    (diag5, diag6) = (env["diag5"], env["diag6"])
    (t2p3, t5p3) = (env["t2p3"], env["t5p3"])
    (big16, tmp_p, small_p, pp) = (
        env["big16"], env["tmp_p"], env["small_p"], env["pp"]
    )

    def dense_conv(n, tag, wT, srcs, emit_out):
        """out[m] = sum_k wT[k][:,m] @ srcs[k], weight-stationary over
        chunks; emit_out(m, ch, ps) evicts one chunk."""
        for m in range(G):
            ps = [
                pp.tile([128, CHF], F32, name=f"ps{tag}{n}{m}{ch}", tag="ps")
                for ch in range(NCH)
            ]
            for k in range(G):
                for ch in range(NCH):
                    nc.tensor.matmul(
                        ps[ch][:],
                        wT[k][:, 128 * m : 128 * (m + 1)],
                        srcs[k][:, CHF * ch : CHF * (ch + 1)],
                        start=(k == 0),
                        stop=(k == G - 1),
                    )
            for ch in range(NCH):
                emit_out(m, ch, ps[ch])

    def dw_conv(n, tag, g, pe_taps, dve_taps, act_taps, diag, wt, view, outv3):
        """Depthwise conv for one channel group.

        view(dy_or_jy, dx_or_jx) -> (row_off, col_off) into the padded
        source; outv3 = [128, 56, 56] output view (interior of the conv
        output). PE taps accumulate in PSUM (weight-stationary) and ACT
        evicts; DVE/ACT hybrid taps are added in place afterwards."""
        src3, widx = view
        ps = [
            pp.tile([128, CHF], F32, name=f"ps{tag}{n}{g}{ch}", tag="ps")
            for ch in range(NCH)
        ]
        for i, t in enumerate(pe_taps):
            r, c = widx(t)
            for ch in range(NCH):
                r0 = CH_ROWS * ch
                nc.tensor.matmul(
                    ps[ch][:],
                    diag[(g,) + t][:],
                    src3[:, r + r0 : r + r0 + CH_ROWS, c : c + W],
                    start=(i == 0),
                    stop=(i == len(pe_taps) - 1),
                )
        for ch in range(NCH):
            r0 = CH_ROWS * ch
            nc.scalar.activation(
                outv3[:, r0 : r0 + CH_ROWS, :], ps[ch][:], AF.Copy
            )
        # hybrid + pair taps: mul into dense tmp, then in-place add
        for t in act_taps:
            r, c = widx(t)
            tmp = tmp_p.tile([128, HW], F16, name=f"tm{tag}{n}{g}a", tag="tmp")
            tmp3 = tmp.rearrange("p (h w) -> p h w", w=W)
            nc.scalar.activation(
                tmp3[:], src3[:, r : r + H, c : c + W], AF.Copy,
                scale=wt[:, t[0] * widx.kw + t[1] : t[0] * widx.kw + t[1] + 1],
            )
            nc.vector.tensor_tensor(outv3[:], outv3[:], tmp3[:], OP.add)
        for t in dve_taps:
            r, c = widx(t)
            tmp = tmp_p.tile([128, HW], F16, name=f"tm{tag}{n}{g}d", tag="tmp")
            tmp3 = tmp.rearrange("p (h w) -> p h w", w=W)
            nc.vector.tensor_scalar_mul(
                tmp3[:], src3[:, r : r + H, c : c + W],
                wt[:, t[0] * widx.kw + t[1] : t[0] * widx.kw + t[1] + 1],
            )
            nc.vector.tensor_tensor(outv3[:], outv3[:], tmp3[:], OP.add)

    for n in range(NS):
        # A) load x (already fp16)
        x16 = []
        for g in range(G):
            xt = big16.tile([128, HW], F16, name=f"x16_{n}_{g}", tag="b16")
            nc.sync.dma_start(out=xt[:], in_=x_d.ap()[n, g])
            x16.append(xt)

        # B) t1 = w1 @ x ; t2 = gelu(t1) -> t2pad interior
        def emit_gelu(m, ch, ps):
            nc.scalar.activation(
                t2p3[m][:, 2 + CH_ROWS * ch : 2 + CH_ROWS * (ch + 1), 2 : 2 + W],
                ps[:],
                AF.Gelu,
            )

        dense_conv(n, "B", w1T, x16, emit_gelu)

        # C) t5 = dw5(t2) -> t5pad interior
        def w5view(t):
            return (t[0], t[1])

        w5view.kw = 5
        for g in range(G):
            dw_conv(
                n, "C", g, PE5, DVE5, ACT5, diag5, w5t[g],
                (t2p3[g], w5view),
                t5p3[g][:, T5R0 : T5R0 + H, T5C0 : T5C0 + W],
            )

        # D) t6 = dw7_dil3(t5) -> dense t6 tiles
        def w6view(t):
            return (3 * t[0], 1 + 3 * t[1])

        w6view.kw = 7
        t6 = []
        for g in range(G):
            t6g = big16.tile([128, HW], F16, name=f"t6_{n}_{g}", tag="b16")
            t6g3 = t6g.rearrange("p (h w) -> p h w", w=W)
            dw_conv(
                n, "D", g, PE7, DVE7, ACT7, diag6, w6t[g],
                (t5p3[g], w6view),
                t6g3[:],
            )
            t6.append(t6g)

        # E) t7 = w7 @ t6 ; t8 = t7 * t2 (in place)
        t8 = [
            big16.tile([128, HW], F16, name=f"t7_{n}_{m}", tag="b16")
            for m in range(G)
        ]
        for m in range(G):
            ps = [
                pp.tile([128, CHF], F32, name=f"psE{n}{m}{ch}", tag="ps")
                for ch in range(NCH)
            ]
            for k in range(G):
                for ch in range(NCH):
                    nc.tensor.matmul(
                        ps[ch][:],
                        w7T[k][:, 128 * m : 128 * (m + 1)],
                        t6[k][:, CHF * ch : CHF * (ch + 1)],
                        start=(k == 0),
                        stop=(k == G - 1),
                    )
            for ch in range(NCH):
                nc.scalar.activation(
                    t8[m][:, CHF * ch : CHF * (ch + 1)], ps[ch][:], AF.Copy
                )
            t7m3 = t8[m].rearrange("p (h w) -> p h w", w=W)
            nc.vector.tensor_tensor(
                t7m3[:],
                t7m3[:],
                t2p3[m][:, 2 : 2 + H, 2 : 2 + W],
                OP.mult,
            )

        # F) t9 = w9 @ t8 ; out = x + t9 (residual read back in fp16)
        for m in range(G):
            ps = [
                pp.tile([128, CHF], F32, name=f"psF{n}{m}{ch}", tag="ps")
                for ch in range(NCH)
            ]
            for k in range(G):
                for ch in range(NCH):
                    nc.tensor.matmul(
                        ps[ch][:],
                        w9T[k][:, 128 * m : 128 * (m + 1)],
                        t8[k][:, CHF * ch : CHF * (ch + 1)],
                        start=(k == 0),
                        stop=(k == G - 1),
                    )
            for ch in range(NCH):
                res = small_p.tile([128, CHF], F16, name=f"rs{n}{m}{ch}", tag="res")
                nc.sync.dma_start(
                    out=res[:], in_=x_d.ap()[n, m, :, CHF * ch : CHF * (ch + 1)]
                )
                ost = small_p.tile([128, CHF], F32, name=f"os{n}{m}{ch}", tag="ost")
                nc.vector.tensor_tensor(ost[:], ps[ch][:], res[:], OP.add)
                nc.sync.dma_start(
                    out=o_d.ap()[n, m, :, CHF * ch : CHF * (ch + 1)], in_=ost[:]
                )


_NC_CACHE = None


def _get_nc():
    global _NC_CACHE
    if _NC_CACHE is None:
        _NC_CACHE = _build_program()
    return _NC_CACHE


def _prep_shared_inputs(w1, w5, w6, w7, w9):
    def lhsT(w):
        return np.ascontiguousarray(np.asarray(w, np.float32).T).astype(np.float16).reshape(G, 128, C)

    return {
        "w1T": lhsT(w1),
        "w7T": lhsT(w7),
        "w9T": lhsT(w9),
        "w5t": np.asarray(w5, np.float32).reshape(C, 25).reshape(G, 128, 25),
        "w6t": np.asarray(w6, np.float32).reshape(C, 49).reshape(G, 128, 49),
        "ident": np.eye(128, dtype=np.float16),
    }


def _make_in_maps(x, w1, w5, w6, w7, w9):
    x16 = np.asarray(x, np.float32).astype(np.float16)
    N = x16.shape[0]
    assert N == N_CORES * NS
    shared = _prep_shared_inputs(w1, w5, w6, w7, w9)
    xs = x16.reshape(N_CORES, NS, G, 128, HW)
    return [{"x16": np.ascontiguousarray(xs[i]), **shared} for i in range(N_CORES)]


def kernel(x, w1, w5, w6, w7, w9, _trace=False, _tmpdir=None):
    in_maps = _make_in_maps(x, w1, w5, w6, w7, w9)
    nc = _get_nc()
    res = run_bass_kernel_spmd(
        nc, in_maps, core_ids=list(range(N_CORES)), trace=_trace, tmpdir=_tmpdir
    )
    outs = [res.results[i]["out"] for i in range(N_CORES)]
    out = np.stack(outs, axis=0).reshape(x.shape[0], C, H, W)
    if _trace:
        kernel.last_exec_time_ns = res.exec_time_ns
        kernel.last_results = res
    return out


# revision 9
# speedup vs baseline: 1.6381x; 1.2141x over previous
"""Trainium2 Bass kernel for the dense_cnn problem:

    t1 = conv1x1(x, w1); t2 = gelu(t1)
    t5 = dwconv5x5(t2, w5, pad=2)
    t6 = dwconv7x7_dil3(t5, w6, pad=9)
    t7 = conv1x1(t6, w7); t8 = t7 * t2; t9 = conv1x1(t8, w9)
    out = x + t9

Sharding: data-parallel over batch N=32 across 8 cores (4 samples/core).

Per-core schedule (fp16 datapath, fp32 PSUM accumulation):
  - x shipped as fp16 (residual also read back in fp16).
  - 1x1 convs: dense PE matmuls (lhsT = W.T in fp16), 7 chunks of 8 rows
    (N=448) per 128-channel group.
  - depthwise convs, split per tap between PE and DVE:
      PE taps: diagonal-matrix matmuls accumulating in PSUM; the scalar
      engine evicts the PSUM partial sum into the conv's output buffer.
      DVE taps: fused scalar_tensor_tensor FMAs (out = src*w + out)
      added in-place on top of the evicted partial sum. Padded buffers
      are laid out so every DVE operand view is 4-byte aligned (2x mode).
  - dw5: 14 PE taps (all odd-dx + 4 even), 11 DVE taps (even dx).
    dw7: 28 PE taps (even jx), 21 DVE taps (odd jx).
"""

import numpy as np

import concourse.bass as bass
import concourse.mybir as mybir
from concourse.tile import TileContext
from concourse.bass_utils import run_bass_kernel_spmd

# ---------------------------------------------------------------------------
# Workaround: this walrus build rejects >N sem waits on the TileContext tail
# drain ("Too many sync wait commands"). Split them one-per-drain.
from concourse.vector_clock import ScopedClock, VectorClock


def _drain_and_barrier_split(self, tick_clock, wait_clock):
    vc = tick_clock.global_clock
    for proc in range(len(vc)):
        tick = vc[proc]
        if tick <= 0:
            continue
        d = self.nc.sync.drain()
        req = ScopedClock({None: VectorClock([0] * len(vc))})
        req.require_at_least(None, proc, tick)
        wait_clock.add_sem_waits(d.ins, req)
    self.nc.all_engine_barrier()
    assert self.sems is not None
    popped = self.nc._tile_sem_poison_stack.pop()
    assert popped is self._sem_poison
    self.nc.clear_and_free_semaphores(list(self.sems.allocated().values()))
    self.nc.all_engine_barrier()


TileContext._drain_and_barrier = _drain_and_barrier_split

# This walrus build also rejects >1 sem wait on regular engine instructions.
# Post-process the serialized BIR: hoist excess waits onto same-engine NoOps
# inserted right before the instruction (engines execute block instructions
# in order, so waiting earlier on the same engine is equivalent).
import json as _json

_orig_to_json_bytes = bass.Bass.to_json_bytes


def _to_json_bytes_split_waits(self):
    d = _json.loads(_orig_to_json_bytes(self))
    ctr = 0
    for fn in d.get("functions", []):
        for blk in fn.get("blocks", []):
            insts = blk.get("instructions", [])
            out = []
            for inst in insts:
                si = inst.get("sync_info")
                waits = (si or {}).get("on_wait") or []
                if len(waits) > 1:
                    for w in waits[:-1]:
                        out.append({
                            "debug": inst.get("debug", 0),
                            "engine": inst["engine"],
                            "ins": [],
                            "outs": [],
                            "name": f"{inst['name']}_hw{ctr}",
                            "opcode": "NoOp",
                            "sync_info": {"on_wait": [w], "on_update": []},
                        })
                        ctr += 1
                    si["on_wait"] = waits[-1:]
                out.append(inst)
            blk["instructions"] = out
    return _json.dumps(d).encode()


bass.Bass.to_json_bytes = _to_json_bytes_split_waits
# ---------------------------------------------------------------------------

F16 = mybir.dt.float16
F32 = mybir.dt.float32
AF = mybir.ActivationFunctionType
OP = mybir.AluOpType

N_CORES = 8
NS = 4              # samples per core
C, H, W = 384, 56, 56
G = 3               # channel groups of 128
HW = H * W          # 3136
# t2 padded buffer: pad 2 on all sides -> [60, 60]
T2H, T2W = 60, 60
# t5 padded buffer: pad 9 rows top/bottom, 10 cols left (so the interior
# starts on an even element offset -> 4B-aligned fp16 DVE views), width
# padded to 76 so the row stride stays even.
T5H, T5W = 74, 76
T5R0, T5C0 = 9, 10  # interior origin
CH_ROWS = 7         # output rows per PSUM chunk
NCH = H // CH_ROWS  # 8 chunks
CHF = CH_ROWS * W   # 392 free elems per chunk
DV_ROWS = 28        # output rows per DVE half
NDV = H // DV_ROWS  # 2

# Depthwise tap assignment, three lanes:
#   PE:  diagonal-matrix matmuls accumulating in PSUM (weight-stationary,
#        7 chunks per tap).
#   DVE pair: ts_mul (tap -> dense tmp) + tensor_tensor add in place on
#        the conv output. The mul needs a 4B-aligned source view:
#        dw5 -> even dx; dw7 -> odd jx (col offset 1+3jx even).
#   ACT hybrid: scalar-engine copy with per-partition scale (any
#        alignment) -> dense tmp, + DVE add in place.
# dw5 view (into t2pad): out(y,x) += w[dy,dx] * t2pad[y+dy, x+dx]
# dw7 view (into t5pad): out(y,x) += w[jy,jx] * t5pad[y+3jy, 1+3jx + x]
_EVEN5 = [(dy, dx) for dy in range(5) for dx in range(5) if dx % 2 == 0]
_ODD5 = [(dy, dx) for dy in range(5) for dx in range(5) if dx % 2 == 1]
DVE5 = _EVEN5[:6]          # 6 pair taps (aligned)
ACT5 = _EVEN5[6:9] + _ODD5[:2]    # 5 hybrid taps
PE5 = _EVEN5[9:] + _ODD5[2:]      # 14 PE taps
_ODD7 = [(jy, jx) for jy in range(7) for jx in range(7) if jx % 2 == 1]
_EVEN7 = [(jy, jx) for jy in range(7) for jx in range(7) if jx % 2 == 0]
DVE7 = _ODD7[:7]           # 7 pair taps (aligned)
ACT7 = _ODD7[7:14] + _EVEN7[:4]   # 11 hybrid taps
PE7 = _ODD7[14:] + _EVEN7[4:]     # 31 PE taps
assert len(PE5) + len(DVE5) + len(ACT5) == 25
assert len(PE7) + len(DVE7) + len(ACT7) == 49


def _build_program(n_loop=1):
    nc = bass.Bass("TRN2", target_bir_lowering=False, debug=False)

    x_d = nc.dram_tensor("x16", [NS, G, 128, HW], F16, kind="ExternalInput")
    w1T_d = nc.dram_tensor("w1T", [G, 128, C], F16, kind="ExternalInput")
    w7T_d = nc.dram_tensor("w7T", [G, 128, C], F16, kind="ExternalInput")
    w9T_d = nc.dram_tensor("w9T", [G, 128, C], F16, kind="ExternalInput")
    w5t_d = nc.dram_tensor("w5t", [G, 128, 25], F32, kind="ExternalInput")
    w6t_d = nc.dram_tensor("w6t", [G, 128, 49], F32, kind="ExternalInput")
    id_d = nc.dram_tensor("ident", [128, 128], F16, kind="ExternalInput")
    o_d = nc.dram_tensor("out", [NS, G, 128, HW], F32, kind="ExternalOutput")

    with TileContext(nc) as tc:
        with (
            tc.tile_pool(name="const", bufs=1) as const,
            tc.tile_pool(name="big16", bufs=4) as big16,
            tc.tile_pool(name="half16", bufs=13) as half_p,
            tc.tile_pool(name="pads", bufs=1) as pads,
            tc.tile_pool(name="tmps", bufs=5) as tmp_p,
            tc.tile_pool(name="small", bufs=4) as small_p,
            tc.tile_pool(name="psum", bufs=8, space="PSUM") as pp,
        ):
            # ---- constants -------------------------------------------------
            w1T = [const.tile([128, C], F16, name=f"w1T{k}") for k in range(G)]
            w7T = [const.tile([128, C], F16, name=f"w7T{k}") for k in range(G)]
            w9T = [const.tile([128, C], F16, name=f"w9T{k}") for k in range(G)]
            w5t = [const.tile([128, 25], F32, name=f"w5t{g}") for g in range(G)]
            w6t = [const.tile([128, 49], F32, name=f"w6t{g}") for g in range(G)]
            ident = const.tile([128, 128], F16, name="ident")
            for k in range(G):
                nc.sync.dma_start(out=w1T[k][:], in_=w1T_d.ap()[k])
                nc.sync.dma_start(out=w7T[k][:], in_=w7T_d.ap()[k])
                nc.sync.dma_start(out=w9T[k][:], in_=w9T_d.ap()[k])
                nc.sync.dma_start(out=w5t[k][:], in_=w5t_d.ap()[k])
                nc.sync.dma_start(out=w6t[k][:], in_=w6t_d.ap()[k])
            nc.sync.dma_start(out=ident[:], in_=id_d.ap())

            # Diagonal weight matrices for PE depthwise taps.
            diag5 = {}
            for g in range(G):
                for (dy, dx) in PE5:
                    t = const.tile([128, 128], F16, name=f"d5_{g}_{dy}_{dx}")
                    nc.vector.tensor_scalar_mul(
                        t[:], ident[:], w5t[g][:, 5 * dy + dx : 5 * dy + dx + 1]
                    )
                    diag5[(g, dy, dx)] = t
            diag6 = {}
            for g in range(G):
                for (jy, jx) in PE7:
                    t = const.tile([128, 128], F16, name=f"d6_{g}_{jy}_{jx}")
                    nc.vector.tensor_scalar_mul(
                        t[:], ident[:], w6t[g][:, 7 * jy + jx : 7 * jy + jx + 1]
                    )
                    diag6[(g, jy, jx)] = t

            # ---- padded scratch (zero margins persist across samples) ------
            t2pad = [pads.tile([128, T2H * T2W], F16, name=f"t2p{g}") for g in range(G)]
            t5pad = [pads.tile([128, T5H * T5W], F16, name=f"t5p{g}") for g in range(G)]
            for g in range(G):
                nc.gpsimd.memset(t2pad[g][:], 0.0)
                nc.gpsimd.memset(t5pad[g][:], 0.0)
            t2p3 = [t.rearrange("p (h w) -> p h w", w=T2W) for t in t2pad]
            t5p3 = [t.rearrange("p (h w) -> p h w", w=T5W) for t in t5pad]

            # ---- per-sample program ---------------------------------------
            import contextlib

            loop_cm = (
                tc.For_i(0, n_loop, 1) if n_loop > 1 else contextlib.nullcontext()
            )
            with loop_cm:
                _emit_samples(nc, tc, locals())
    return nc


def _emit_samples(nc, tc, env):
    (x_d, o_d) = (env["x_d"], env["o_d"])
    (w1T, w7T, w9T, w5t, w6t) = (
        env["w1T"], env["w7T"], env["w9T"], env["w5t"], env["w6t"]
    )
    (diag5, diag6) = (env["diag5"], env["diag6"])
    (t2p3, t5p3) = (env["t2p3"], env["t5p3"])
    (big16, half_p, tmp_p, small_p, pp) = (
        env["big16"], env["half_p"], env["tmp_p"], env["small_p"], env["pp"]
    )

    def dense_conv(n, tag, wT, srcs, emit_out):
        """out[m] = sum_k wT[k][:,m] @ srcs[k]; per-chunk PSUM groups;
        srcs entries are (tile, chunk_slicer)."""
        for m in range(G):
            for ch in range(NCH):
                ps = pp.tile([128, CHF], F32, name=f"ps{tag}{n}{m}{ch}", tag="ps")
                for k in range(G):
                    nc.tensor.matmul(
                        ps[:],
                        wT[k][:, 128 * m : 128 * (m + 1)],
                        srcs[k](ch),
                        start=(k == 0),
                        stop=(k == G - 1),
                    )
                emit_out(m, ch, ps)

    def dw_conv(n, tag, g, pe_taps, dve_taps, act_taps, diag, wt, view, out_ch,
                out_half):
        """Depthwise conv for one channel group.

        view = (src3, widx); widx(t) -> (row_off, col_off) into the padded
        source. out_ch(ch) -> [128, 7, 56] eviction view for chunk ch;
        out_half(d) -> [128, 28, 56] in-place add view for half d.
        PE taps accumulate in PSUM and ACT evicts; DVE pair taps and
        ACT-hybrid taps are added in place afterwards, per half."""
        src3, widx = view
        for ch in range(NCH):
            r0 = CH_ROWS * ch
            ps = pp.tile([128, CHF], F32, name=f"ps{tag}{n}{g}{ch}", tag="ps")
            for i, t in enumerate(pe_taps):
                r, c = widx(t)
                nc.tensor.matmul(
                    ps[:],
                    diag[(g,) + t][:],
                    src3[:, r + r0 : r + r0 + CH_ROWS, c : c + W],
                    start=(i == 0),
                    stop=(i == len(pe_taps) - 1),
                )
            nc.scalar.activation(out_ch(ch), ps[:], AF.Copy)
        for d in range(NDV):
            r0 = DV_ROWS * d
            outv = out_half(d)
            for t in act_taps:
                r, c = widx(t)
                tmp = tmp_p.tile([128, DV_ROWS * W], F16,
                                 name=f"tm{tag}{n}{g}{d}a", tag="tmp")
                tmp3 = tmp.rearrange("p (h w) -> p h w", w=W)
                nc.scalar.activation(
                    tmp3[:], src3[:, r + r0 : r + r0 + DV_ROWS, c : c + W],
                    AF.Copy,
                    scale=wt[:, t[0] * widx.kw + t[1] : t[0] * widx.kw + t[1] + 1],
                )
                nc.vector.tensor_tensor(outv, outv, tmp3[:], OP.add)
            for t in dve_taps:
                r, c = widx(t)
                tmp = tmp_p.tile([128, DV_ROWS * W], F16,
                                 name=f"tm{tag}{n}{g}{d}d", tag="tmp")
                tmp3 = tmp.rearrange("p (h w) -> p h w", w=W)
                nc.vector.tensor_scalar_mul(
                    tmp3[:], src3[:, r + r0 : r + r0 + DV_ROWS, c : c + W],
                    wt[:, t[0] * widx.kw + t[1] : t[0] * widx.kw + t[1] + 1],
                )
                nc.vector.tensor_tensor(outv, outv, tmp3[:], OP.add)

    HNCH = NCH // NDV      # chunks per half (4)
    HF = HNCH * CHF        # free elems per half tile (1568)

    def half_slicer(halves):
        def sl(ch):
            return halves[ch // HNCH][:, CHF * (ch % HNCH) : CHF * (ch % HNCH + 1)]
        return sl

    for n in range(NS):
        # A) load x (already fp16)
        x16 = []
        for g in range(G):
            xt = big16.tile([128, HW], F16, name=f"x16_{n}_{g}", tag="b16")
            nc.sync.dma_start(out=xt[:], in_=x_d.ap()[n, g])
            x16.append(xt)

        # B) t1 = w1 @ x ; t2 = gelu(t1) -> t2pad interior
        def emit_gelu(m, ch, ps):
            nc.scalar.activation(
                t2p3[m][:, 2 + CH_ROWS * ch : 2 + CH_ROWS * (ch + 1), 2 : 2 + W],
                ps[:],
                AF.Gelu,
            )

        dense_conv(
            n, "B", w1T,
            [(lambda ch, _x=x: _x[:, CHF * ch : CHF * (ch + 1)]) for x in x16],
            emit_gelu,
        )

        # C) t5 = dw5(t2) -> t5pad interior
        def w5view(t):
            return (t[0], t[1])

        w5view.kw = 5
        for g in range(G):
            dw_conv(
                n, "C", g, PE5, DVE5, ACT5, diag5, w5t[g],
                (t2p3[g], w5view),
                lambda ch, _g=g: t5p3[_g][:, T5R0 + CH_ROWS * ch :
                                          T5R0 + CH_ROWS * (ch + 1),
                                          T5C0 : T5C0 + W],
                lambda d, _g=g: t5p3[_g][:, T5R0 + DV_ROWS * d :
                                         T5R0 + DV_ROWS * (d + 1),
                                         T5C0 : T5C0 + W],
            )

        # D) t6 = dw7_dil3(t5) -> two half tiles per group (top/bottom 28
        #    rows) so E-stage chunks can start as soon as a half is done.
        def w6view(t):
            return (3 * t[0], 1 + 3 * t[1])

        w6view.kw = 7
        t6 = []
        for g in range(G):
            halves = [
                half_p.tile([128, HF], F16, name=f"t6_{n}_{g}_{d}", tag="h16")
                for d in range(NDV)
            ]
            h3 = [h.rearrange("p (h w) -> p h w", w=W) for h in halves]
            dw_conv(
                n, "D", g, PE7, DVE7, ACT7, diag6, w6t[g],
                (t5p3[g], w6view),
                lambda ch, _h3=h3: _h3[ch // HNCH][
                    :, CH_ROWS * (ch % HNCH) : CH_ROWS * (ch % HNCH + 1), :],
                lambda d, _h3=h3: _h3[d][:],
            )
            t6.append(halves)

        # E) t7 = w7 @ t6 ; t8 = t7 * t2 (in place), per half
        t8 = [
            [
                half_p.tile([128, HF], F16, name=f"t7_{n}_{m}_{d}", tag="h16")
                for d in range(NDV)
            ]
            for m in range(G)
        ]

        def emit_t7(m, ch, ps):
            nc.scalar.activation(
                t8[m][ch // HNCH][:, CHF * (ch % HNCH) : CHF * (ch % HNCH + 1)],
                ps[:],
                AF.Copy,
            )

        dense_conv(n, "E", w7T, [half_slicer(h) for h in t6], emit_t7)
        for m in range(G):
            for d in range(NDV):
                t7h3 = t8[m][d].rearrange("p (h w) -> p h w", w=W)
                nc.vector.tensor_tensor(
                    t7h3[:],
                    t7h3[:],
                    t2p3[m][:, 2 + DV_ROWS * d : 2 + DV_ROWS * (d + 1), 2 : 2 + W],
                    OP.mult,
                )

        # F) t9 = w9 @ t8 ; out = x + t9 (residual read back in fp16)
        def emit_out(m, ch, ps):
            res = small_p.tile([128, CHF], F16, name=f"rs{n}{m}{ch}", tag="res")
            nc.sync.dma_start(
                out=res[:], in_=x_d.ap()[n, m, :, CHF * ch : CHF * (ch + 1)]
            )
            ost = small_p.tile([128, CHF], F32, name=f"os{n}{m}{ch}", tag="ost")
            nc.vector.tensor_tensor(ost[:], ps[:], res[:], OP.add)
            nc.sync.dma_start(
                out=o_d.ap()[n, m, :, CHF * ch : CHF * (ch + 1)], in_=ost[:]
            )

        dense_conv(n, "F", w9T, [half_slicer(h) for h in t8], emit_out)


_NC_CACHE = None


def _get_nc():
    global _NC_CACHE
    if _NC_CACHE is None:
        _NC_CACHE = _build_program()
    return _NC_CACHE


def _prep_shared_inputs(w1, w5, w6, w7, w9):
    def lhsT(w):
        return np.ascontiguousarray(np.asarray(w, np.float32).T).astype(np.float16).reshape(G, 128, C)

    return {
        "w1T": lhsT(w1),
        "w7T": lhsT(w7),
        "w9T": lhsT(w9),
        "w5t": np.asarray(w5, np.float32).reshape(C, 25).reshape(G, 128, 25),
        "w6t": np.asarray(w6, np.float32).reshape(C, 49).reshape(G, 128, 49),
        "ident": np.eye(128, dtype=np.float16),
    }


def _make_in_maps(x, w1, w5, w6, w7, w9):
    x16 = np.asarray(x, np.float32).astype(np.float16)
    N = x16.shape[0]
    assert N == N_CORES * NS
    shared = _prep_shared_inputs(w1, w5, w6, w7, w9)
    xs = x16.reshape(N_CORES, NS, G, 128, HW)
    return [{"x16": np.ascontiguousarray(xs[i]), **shared} for i in range(N_CORES)]


def kernel(x, w1, w5, w6, w7, w9, _trace=False, _tmpdir=None):
    in_maps = _make_in_maps(x, w1, w5, w6, w7, w9)
    nc = _get_nc()
    res = run_bass_kernel_spmd(
        nc, in_maps, core_ids=list(range(N_CORES)), trace=_trace, tmpdir=_tmpdir
    )
    outs = [res.results[i]["out"] for i in range(N_CORES)]
    out = np.stack(outs, axis=0).reshape(x.shape[0], C, H, W)
    if _trace:
        kernel.last_exec_time_ns = res.exec_time_ns
        kernel.last_results = res
    return out
